# revision 1
# baseline (speedup 1.0000x reference)
"""Low-rank attention kernel for Trainium2, distributed over 8 NeuronCores.

Math (per batch b):
    u  = q @ Wu            [N, R]
    vp = k @ Wv            [N, R]
    S  = u @ vp.T / sqrt(R)
    out = softmax(S) @ v   [N, D]

Shapes: B=4, N=4096, D=1024, R=32.

Sharding: data-parallel over batch x row-halves -> 8 shards. Core c handles
batch b = c // 2, rows [h*2048, (h+1)*2048) with h = c % 2. Each core gets its
q-shard and the full k/v for its batch. q/k are fed pre-transposed ([D, n]
layout) so every matmul contraction lands on the partition axis with no
on-device transposes.

Per-core device kernel (all matmuls in float32r: full PE rate, ~1e-4 rel err):
  1. uT[R, 2048]  = sum_d Wu[d, :].T qT[d, :]   (K=128 d-tiles, PSUM accum)
     vpT[R, 4096] = sum_d Wv[d, :].T kT[d, :]
  2. flash-style main loop over n-chunks of 256 rows:
       for each m-tile (128 cols): scoresT[m128, n256] = vpT_tile.T @ uT_chunk
       expT = Exp(scoresT / sqrt(R))                       (ScalarE, PSUM->SBUF)
       out_acc[n128, d512] += expT_tile.T @ v_tile         (PSUM accum over m)
       sum_acc[n128, 1]    += expT_tile.T @ ones
     out = out_acc * (1 / sum_acc)   (softmax normalization folded at the end)
"""

import numpy as np

B, N, D, R = 4, 4096, 1024, 32
NLOC = N // 2            # rows per core
RSCALE = float(1.0 / np.sqrt(np.float32(R)))

N_CHUNK = 256            # rows of scores computed per PSUM round
M_TILE = 128             # contraction tile for the AV matmul
D_HALF = 512             # PSUM bank width in fp32

LAST_RESULT = None       # test.py reads exec_time_ns etc. from here


def _build():
    from concourse import bacc, mybir
    from concourse.tile import TileContext

    f32 = mybir.dt.float32
    f32r = mybir.dt.float32r
    f16 = mybir.dt.float16
    EXP = mybir.ActivationFunctionType.Exp
    COPY = mybir.ActivationFunctionType.Copy

    nc = bacc.Bacc("TRN2", target_bir_lowering=False)

    qT = nc.dram_tensor("qT", [D, NLOC], f32r, kind="ExternalInput")
    kT = nc.dram_tensor("kT", [D, N], f32r, kind="ExternalInput")
    v = nc.dram_tensor("v", [N, D], f16, kind="ExternalInput")
    wu = nc.dram_tensor("wu", [D, R], f32r, kind="ExternalInput")
    wv = nc.dram_tensor("wv", [D, R], f32r, kind="ExternalInput")
    o = nc.dram_tensor("o", [NLOC, D], f32, kind="ExternalOutput")

    DT = D // 128         # 8 d-tiles
    NQ = NLOC // 1024     # 2 column-halves of qT
    MQ = N // 1024        # 4 column-quarters of kT
    NCH = NLOC // N_CHUNK  # 8 main-loop chunks
    MT = N // M_TILE      # 32 m tiles
    VG = 8                # v row-groups of 512
    VPG = N // VG // 128  # 4 m-tiles per v group

    with TileContext(nc) as tc:
        with tc.tile_pool(name="singles", bufs=1) as singles, \
             tc.tile_pool(name="stream", bufs=20) as stream, \
             tc.tile_pool(name="vpool", bufs=VG) as vpool, \
             tc.tile_pool(name="expp", bufs=6) as expp, \
             tc.tile_pool(name="outp", bufs=3) as outp, \
             tc.tile_pool(name="rpool", bufs=4) as rpool, \
             tc.tile_pool(name="pacc", bufs=4, space="PSUM") as pacc, \
             tc.tile_pool(name="pscore", bufs=3, space="PSUM") as pscore, \
             tc.tile_pool(name="psums", bufs=1, space="PSUM") as psums:

            # ---- constants / projection weights ----
            wu_sb = singles.tile([128, DT, R], f32r, tag="wu")
            nc.sync.dma_start(out=wu_sb, in_=wu.rearrange("(t p) r -> p t r", p=128))
            wv_sb = singles.tile([128, DT, R], f32r, tag="wv")
            nc.sync.dma_start(out=wv_sb, in_=wv.rearrange("(t p) r -> p t r", p=128))
            ones = singles.tile([128, 2], f16, tag="ones")
            nc.vector.memset(ones, 1.0)

            uT = singles.tile([R, NLOC], f32r, tag="uT")
            vpT = singles.tile([R, N], f32r, tag="vpT")

            # ---- phase 1a: uT = Wu.T @ q  (per d-tile: wu_sb[:,t,:].T @ qT_t)
            def load_qt(h):
                tiles = []
                for t in range(DT):
                    tile = stream.tile([128, 1024], f32r, tag="stream",
                                       name=f"qt{h}_{t}")
                    nc.sync.dma_start(
                        out=tile, in_=qT[t * 128:(t + 1) * 128,
                                         h * 1024:(h + 1) * 1024])
                    tiles.append(tile)
                return tiles

            qt = {}
            for t, tile in enumerate(load_qt(0)):
                qt[(t, 0)] = tile
            def u_chunk(c):
                h, off = c // 2, (c % 2) * 512
                pu = pscore.tile([R, 512], f32, tag="scores", name=f"pu{c}")
                for t in range(DT):
                    nc.tensor.matmul(pu, lhsT=wu_sb[:, t, :],
                                     rhs=qt[(t, h)][:, off:off + 512],
                                     start=(t == 0), stop=(t == DT - 1))
                nc.vector.tensor_copy(out=uT[:, c * 512:(c + 1) * 512], in_=pu)

            for c in (0, 1):
                u_chunk(c)

            # ---- v tiles, interleaved with kT quarters so neither starves
            v_sb = [None] * VG

            def load_v(g):
                vt = vpool.tile([128, VPG, D], f16, tag="v", name=f"v{g}")
                nc.sync.dma_start(
                    out=vt, in_=v[g * 512:(g + 1) * 512, :].rearrange(
                        "(t p) d -> p t d", p=128))
                v_sb[g] = vt

            load_v(0)
            load_v(1)

            # ---- phase 1b: vpT = Wv.T @ k
            for qtr in range(MQ):
                kt = []
                for t in range(DT):
                    tile = stream.tile([128, 1024], f32r, tag="stream")
                    nc.sync.dma_start(
                        out=tile, in_=kT[t * 128:(t + 1) * 128,
                                         qtr * 1024:(qtr + 1) * 1024])
                    kt.append(tile)
                if qtr < 3:
                    load_v(2 + 2 * qtr)
                    load_v(3 + 2 * qtr)
                for c2 in range(2):
                    pv = pscore.tile([R, 512], f32, tag="scores")
                    for t in range(DT):
                        nc.tensor.matmul(pv, lhsT=wv_sb[:, t, :],
                                         rhs=kt[t][:, c2 * 512:c2 * 512 + 512],
                                         start=(t == 0), stop=(t == DT - 1))
                    off = qtr * 1024 + c2 * 512
                    nc.vector.tensor_copy(out=vpT[:, off:off + 512], in_=pv)

            for t, tile in enumerate(load_qt(1)):
                qt[(t, 1)] = tile
            for c in (2, 3):
                u_chunk(c)

            # ---- phase 2: flash-style scores/softmax/AV ----
            # software-pipelined: scores/exp for m-tile mt+1 are issued before
            # the AV matmuls of m-tile mt, so ScalarE exp latency hides under
            # the previous tile's AV work on the PE.
            for ch in range(NCH):
                accs = [pacc.tile([128, D_HALF], f32, tag="acc", name=f"acc{ch}_{i}")
                        for i in range(4)]
                # both sums accumulators share one bank: start=True clears
                # has_written bank-wide, so ONLY sums[0]'s first matmul carries
                # start=True (issued before any other write to the bank); the
                # cleared has_written makes sums[1]'s first start=False matmul
                # overwrite rather than accumulate stale data
                sums_t = psums.tile([128, 4], f32, tag="sums", name=f"sum{ch}")
                sums = [sums_t[:, 0:2], sums_t[:, 2:4]]

                def scores_exp(mt):
                    ps = pscore.tile([128, N_CHUNK], f32, tag="scores",
                                     name=f"ps{ch}_{mt}")
                    nc.tensor.matmul(
                        ps, lhsT=vpT[:, mt * 128:(mt + 1) * 128],
                        rhs=uT[:, ch * N_CHUNK:(ch + 1) * N_CHUNK],
                        start=True, stop=True)
                    ex = expp.tile([128, N_CHUNK], f16, tag="ex",
                                   name=f"ex{ch}_{mt}")
                    nc.scalar.activation(out=ex, in_=ps, func=EXP, scale=RSCALE)
                    return ex

                ex_q = [scores_exp(0), scores_exp(1)]
                for mt in range(MT):
                    ex = ex_q.pop(0)
                    if mt + 2 < MT:
                        ex_q.append(scores_exp(mt + 2))
                    g, tg = mt // VPG, mt % VPG
                    first, last = (mt == 0), (mt == MT - 1)
                    for j in range(2):
                        lhs = ex[:, j * 128:(j + 1) * 128]
                        nc.tensor.matmul(accs[2 * j], lhsT=lhs,
                                         rhs=v_sb[g][:, tg, 0:D_HALF],
                                         start=first, stop=last)
                        nc.tensor.matmul(accs[2 * j + 1], lhsT=lhs,
                                         rhs=v_sb[g][:, tg, D_HALF:D],
                                         start=first, stop=last)
                        nc.tensor.matmul(sums[j], lhsT=lhs, rhs=ones,
                                         start=(first and j == 0), stop=last,
                                         skip_group_check=True)
                # normalize on DVE (keeps ScalarE free for next chunk's exp)
                for j in range(2):
                    rc = rpool.tile([128, 1], f32, tag="rc", name=f"rc{ch}_{j}")
                    nc.vector.reciprocal(rc, sums[j][:, 0:1])
                    ob = outp.tile([128, D], f32, tag="ob", name=f"ob{ch}_{j}")
                    nc.vector.tensor_scalar_mul(ob[:, 0:D_HALF], accs[2 * j], rc)
                    nc.vector.tensor_scalar_mul(ob[:, D_HALF:D], accs[2 * j + 1], rc)
                    row = ch * N_CHUNK + j * 128
                    nc.sync.dma_start(out=o[row:row + 128, :], in_=ob)

    nc.finalize()
    return nc


def kernel(q, k, v, Wu, Wv):
    global LAST_RESULT
    from concourse import bass_utils

    nc = _build()

    kTs = [np.ascontiguousarray(k[b].T) for b in range(B)]
    vs = [np.ascontiguousarray(v[b]).astype(np.float16) for b in range(B)]
    in_maps = []
    for core in range(8):
        b, h = core // 2, core % 2
        in_maps.append({
            "qT": np.ascontiguousarray(q[b].T[:, h * NLOC:(h + 1) * NLOC]),
            "kT": kTs[b],
            "v": vs[b],
            "wu": np.ascontiguousarray(Wu),
            "wv": np.ascontiguousarray(Wv),
        })

    res = bass_utils.run_bass_kernel_spmd(nc, in_maps, core_ids=list(range(8)))
    LAST_RESULT = res

    out = np.empty((B, N, D), dtype=np.float32)
    for core in range(8):
        b, h = core // 2, core % 2
        out[b, h * NLOC:(h + 1) * NLOC, :] = res.results[core]["o"]
    return out



# revision 7
# speedup vs baseline: 1.6985x; 1.6985x over previous
"""Low-rank attention kernel for Trainium2, distributed over 8 NeuronCores.

Math (per batch b):
    u  = q @ Wu            [N, R]
    vp = k @ Wv            [N, R]
    S  = u @ vp.T / sqrt(R)
    out = softmax(S) @ v   [N, D]

Shapes: B=4, N=4096, D=1024, R=32.

Sharding: data-parallel over batch x row-halves -> 8 shards. Core c handles
batch b = c // 2, rows [h*2048, (h+1)*2048) with h = c % 2. Each core gets its
q-shard and the full k/v for its batch.

All device tensors are float16 (inputs cast on host): every matmul is f16 so
the compiler's fast-weight-load path stays enabled, and input DMA is half of
the f32 version. PSUM accumulation is f32 throughout, so the only precision
losses are the f16 input rounding and the f16 exp tiles (~1e-3 rel).

Per-core device kernel:
  1. uT[128, 2048] / vpT[128, 4096] = projections, with Wu/Wv pre-replicated
     4x along the rank axis on the host so uT/vpT carry 4 copies of the
     32 rank rows at partition offsets 0/32/64/96.
  2. flash-style main loop over 8 chunks of 256 query rows:
       scores: m-tiles computed 4 at a time with row-packed K=32 matmuls
               (tile_position=(32i,0)) -> ~4x fewer PE-serial score cycles
       exp:    ScalarE activation per m-tile pair ([128, 512] tiles)
       sums:   DVE accumulates exp tiles into S2[128,512]; 4 tiny ones-matmuls
               per chunk turn S2 into the softmax denominators (keeps the
               512 per-m-tile ones-matmuls of the naive version off the PE)
       AV:     acc[128n, 512d] += exT.T @ v tiles, PSUM accumulation over m
       out = acc * (1/sums), f16, DMA'd out (host casts back to f32)
"""

import numpy as np

B, N, D, R = 4, 4096, 1024, 32
NLOC = N // 2            # rows per core
RSCALE = float(1.0 / np.sqrt(np.float32(R)))

N_CHUNK = 256            # query rows per PSUM round
NCH = NLOC // N_CHUNK    # 8 chunks
NPAIR = N // 256         # 16 m-tile pairs per chunk
DT = D // 128            # 8 d-tiles

LAST_RESULT = None       # test.py reads exec_time_ns etc. from here


def _build():
    from concourse import bacc, mybir
    from concourse.tile import TileContext

    f16 = mybir.dt.float16
    f32 = mybir.dt.float32
    EXP = mybir.ActivationFunctionType.Exp
    COPY = mybir.ActivationFunctionType.Copy
    ADD = mybir.AluOpType.add

    nc = bacc.Bacc("TRN2", target_bir_lowering=False)

    qT = nc.dram_tensor("qT", [D, NLOC], f16, kind="ExternalInput")
    kT = nc.dram_tensor("kT", [D, N], f16, kind="ExternalInput")
    v = nc.dram_tensor("v", [N, D], f16, kind="ExternalInput")
    wu = nc.dram_tensor("wu", [D, 128], f16, kind="ExternalInput")  # Wu tiled 4x
    wv = nc.dram_tensor("wv", [D, 128], f16, kind="ExternalInput")  # Wv tiled 4x
    o = nc.dram_tensor("o", [NLOC, D], f16, kind="ExternalOutput")

    with TileContext(nc) as tc:
        with tc.tile_pool(name="singles", bufs=1) as singles, \
             tc.tile_pool(name="ktp", bufs=3) as ktp, \
             tc.tile_pool(name="vpool", bufs=8) as vpool, \
             tc.tile_pool(name="expp", bufs=7) as expp, \
             tc.tile_pool(name="saccp", bufs=2) as saccp, \
             tc.tile_pool(name="outp", bufs=3) as outp, \
             tc.tile_pool(name="rpool", bufs=4) as rpool, \
             tc.tile_pool(name="pacc", bufs=4, space="PSUM") as pacc, \
             tc.tile_pool(name="pscore", bufs=2, space="PSUM") as pscore:

            # ---- weights + constants ----
            wu_sb = singles.tile([128, DT, 128], f16, tag="wu")
            nc.sync.dma_start(out=wu_sb, in_=wu.rearrange("(t p) r -> p t r", p=128))
            wv_sb = singles.tile([128, DT, 128], f16, tag="wv")
            nc.sync.dma_start(out=wv_sb, in_=wv.rearrange("(t p) r -> p t r", p=128))
            ones = singles.tile([128, 1], f16, tag="ones")
            nc.vector.memset(ones, 1.0)

            uT = singles.tile([128, NLOC], f16, tag="uT")
            vpT = singles.tile([128, N], f16, tag="vpT")

            # kT quarter 0 first: vp-proj of quarter 0 gates the main loop
            kts = [None] * 4

            def load_kt(qtr):
                t_ = ktp.tile([128, DT, 1024], f16, tag="kt", name=f"kt{qtr}")
                for t in range(DT):
                    nc.sync.dma_start(
                        out=t_[:, t, :],
                        in_=kT[t * 128:(t + 1) * 128,
                              qtr * 1024:(qtr + 1) * 1024])
                kts[qtr] = t_

            load_kt(0)

            qt = singles.tile([128, DT, NLOC], f16, tag="qt")
            for t in range(DT):
                nc.sync.dma_start(out=qt[:, t, :], in_=qT[t * 128:(t + 1) * 128, :])

            v_sb = [None] * 8

            def load_v(g):
                vt = vpool.tile([128, 4, D], f16, tag="v", name=f"v{g}")
                for t in range(4):
                    nc.sync.dma_start(
                        out=vt[:, t, :],
                        in_=v[g * 512 + t * 128:g * 512 + (t + 1) * 128, :])
                v_sb[g] = vt

            load_v(0)
            load_v(1)
            load_kt(1)

            def vp_proj(qtr):
                for c2 in range(2):
                    pv = pscore.tile([128, 512], f32, tag="score",
                                     name=f"pv{qtr}_{c2}")
                    for t in range(DT):
                        nc.tensor.matmul(pv, lhsT=wv_sb[:, t, :],
                                         rhs=kts[qtr][:, t, c2 * 512:(c2 + 1) * 512],
                                         start=(t == 0), stop=(t == DT - 1))
                    off = qtr * 1024 + c2 * 512
                    nc.vector.tensor_copy(out=vpT[:, off:off + 512], in_=pv)

            def u_proj(c):
                pu = pscore.tile([128, 512], f32, tag="score", name=f"pu{c}")
                for t in range(DT):
                    nc.tensor.matmul(pu, lhsT=wu_sb[:, t, :],
                                     rhs=qt[:, t, c * 512:(c + 1) * 512],
                                     start=(t == 0), stop=(t == DT - 1))
                # ScalarE copy keeps the DVE free for the vp copies
                nc.scalar.activation(out=uT[:, c * 512:(c + 1) * 512], in_=pu,
                                     func=COPY)

            vp_proj(0)
            u_proj(0)
            load_v(2)
            load_v(3)
            load_kt(2)

            # ---- main loop ----
            def chunk(ch, interleave):
                accs = [pacc.tile([128, 512], f32, tag="acc", name=f"acc{ch}_{i}")
                        for i in range(4)]
                S2 = saccp.tile([128, 2, 256], f16, tag="sacc", name=f"S2_{ch}")
                exq = {}
                rcs = []
                sums_box = []

                def scores_exp(g):
                    # 4 m-tiles of scores as one row-packed group: K=32 matmuls
                    # in 4 concurrent row-strips of the PE array. Concurrent
                    # packed matmuls must NOT share a PSUM bank (hangs the
                    # device), so each writes its own bank of a 2-bank tile;
                    # the exp activation reads both banks in one strided AP.
                    ps = [pscore.tile([128, 2, 512], f32, tag="score",
                                      name=f"ps{ch}_{g}_{h}") for h in range(2)]
                    for i in range(4):
                        mt = 4 * g + i
                        nc.tensor.matmul(
                            ps[i // 2][:, i % 2, 0:N_CHUNK],
                            lhsT=vpT[32 * i:32 * (i + 1), mt * 128:(mt + 1) * 128],
                            rhs=uT[32 * i:32 * (i + 1),
                                   ch * N_CHUNK:(ch + 1) * N_CHUNK],
                            start=True, stop=True,
                            tile_position=(32 * i, 0),
                            skip_group_check=True)
                    for h in range(2):
                        p = 2 * g + h
                        ex = expp.tile([128, 2, 256], f16, tag="ex",
                                       name=f"ex{ch}_{p}")
                        nc.scalar.activation(out=ex, in_=ps[h][:, :, 0:N_CHUNK],
                                             func=EXP, scale=RSCALE)
                        exq[p] = ex

                def s2_add(p):
                    # running DVE sum of exp tiles; cols [0:256) even m-tiles,
                    # [256:512) odd, reduced to denominators by the 4
                    # ones-matmuls below
                    if p == 0:
                        nc.vector.tensor_copy(out=S2, in_=exq[0])
                    else:
                        nc.vector.tensor_tensor(S2, S2, exq[p], ADD)

                scores_exp(0)
                scores_exp(1)
                s2_add(0)
                s2_add(1)

                for p in range(NPAIR):
                    fn = interleave.get(p)
                    if fn:
                        fn()
                    if p % 2 == 0 and p + 4 < NPAIR:
                        scores_exp((p + 4) // 2)
                    if p + 2 < NPAIR:
                        s2_add(p + 2)
                    if p == NPAIR - 3:
                        # S2 is fully issued; reduce the 128 partial sums per
                        # column with ones-matmuls. These are sequential (not
                        # row-packed), so the shared-bank start=False trick is
                        # safe here; only the first matmul carries start=True.
                        sums_t = pscore.tile([128, 2], f32, tag="score",
                                             name=f"sums{ch}")
                        sums_box.append(sums_t)
                        nc.tensor.matmul(sums_t[:, 0:1], lhsT=S2[:, 0, 0:128],
                                         rhs=ones, start=True, stop=False,
                                         skip_group_check=True)
                        nc.tensor.matmul(sums_t[:, 0:1], lhsT=S2[:, 1, 0:128],
                                         rhs=ones, start=False, stop=True,
                                         skip_group_check=True)
                        nc.tensor.matmul(sums_t[:, 1:2], lhsT=S2[:, 0, 128:256],
                                         rhs=ones, start=False, stop=False,
                                         skip_group_check=True)
                        nc.tensor.matmul(sums_t[:, 1:2], lhsT=S2[:, 1, 128:256],
                                         rhs=ones, start=False, stop=True,
                                         skip_group_check=True)
                        for j in range(2):
                            rc = rpool.tile([128, 1], f32, tag="rc",
                                            name=f"rc{ch}_{j}")
                            nc.vector.reciprocal(rc, sums_t[:, j:j + 1])
                            rcs.append(rc)
                    ex = exq.pop(p)
                    for i in range(2):
                        mt = 2 * p + i
                        g_, tg = mt // 4, mt % 4
                        first, last = (mt == 0), (mt == 2 * NPAIR - 1)
                        for j in range(2):
                            lhs = ex[:, i, j * 128:(j + 1) * 128]
                            nc.tensor.matmul(accs[2 * j], lhsT=lhs,
                                             rhs=v_sb[g_][:, tg, 0:512],
                                             start=first, stop=last)
                            nc.tensor.matmul(accs[2 * j + 1], lhsT=lhs,
                                             rhs=v_sb[g_][:, tg, 512:1024],
                                             start=first, stop=last)

                for j in range(2):
                    ob = outp.tile([128, D], f16, tag="ob", name=f"ob{ch}_{j}")
                    nc.vector.tensor_scalar_mul(ob[:, 0:512], accs[2 * j], rcs[j])
                    nc.vector.tensor_scalar_mul(ob[:, 512:1024], accs[2 * j + 1],
                                                rcs[j])
                    row = ch * N_CHUNK + j * 128
                    nc.sync.dma_start(out=o[row:row + 128, :], in_=ob)

            # chunk 0 interleaves the remaining projections/DMA so the PE
            # never waits on the full kT: pack(g) only needs vpT quarter g//2,
            # issued just in time
            def c0_p0():
                load_kt(3)
                load_v(4)
                load_v(5)
                vp_proj(1)
                u_proj(1)

            def c0_p4():
                load_v(6)
                load_v(7)
                vp_proj(2)

            chunk(0, {0: c0_p0, 4: c0_p4, 8: lambda: vp_proj(3)})
            chunk(1, {})
            chunk(2, {0: lambda: u_proj(2)})
            chunk(3, {})
            chunk(4, {0: lambda: u_proj(3)})
            for ch in range(5, NCH):
                chunk(ch, {})

    nc.finalize()
    return nc


def kernel(q, k, v, Wu, Wv):
    global LAST_RESULT
    from concourse import bass_utils

    nc = _build()

    wu_rep = np.ascontiguousarray(
        np.concatenate([Wu] * 4, axis=1).astype(np.float16))
    wv_rep = np.ascontiguousarray(
        np.concatenate([Wv] * 4, axis=1).astype(np.float16))
    kTs = [np.ascontiguousarray(k[b].T.astype(np.float16)) for b in range(B)]
    vs = [np.ascontiguousarray(v[b].astype(np.float16)) for b in range(B)]
    qTs = [np.ascontiguousarray(q[b].T.astype(np.float16)) for b in range(B)]
    in_maps = []
    for core in range(8):
        b, h = core // 2, core % 2
        in_maps.append({
            "qT": np.ascontiguousarray(qTs[b][:, h * NLOC:(h + 1) * NLOC]),
            "kT": kTs[b],
            "v": vs[b],
            "wu": wu_rep,
            "wv": wv_rep,
        })

    res = bass_utils.run_bass_kernel_spmd(nc, in_maps, core_ids=list(range(8)))
    LAST_RESULT = res

    out = np.empty((B, N, D), dtype=np.float32)
    for core in range(8):
        b, h = core // 2, core % 2
        out[b, h * NLOC:(h + 1) * NLOC, :] = res.results[core]["o"].astype(
            np.float32)
    return out
